# revision 13
# baseline (speedup 1.0000x reference)
"""GroupedQueryAttentionLayer on 8 trn2 NeuronCores (Bass/Tile, SPMD).

Sharding: data-parallel over query rows; no collectives. Core i handles
batch b = i//4, query rows q0 = (i%4)*512 .. +512. Each core recomputes
its batch's K/V projection (cheap vs. attention); outputs are disjoint
row-slices of the final [2, 2048, 1024].

Host-prepared per-core layouts (transposes/casts are part of sharding):
  XT   [128, 8, 2048] fp8e4 : X[b].T k-chunked (XT[p,c,t] = X[b,t,c*128+p])
  XTQ  [128, 8,  512] fp8e4 : XT columns q0..q0+512 (this core's queries)
  XTR  [128, 8,  512] bf16  : same as XTQ in bf16 (residual, added in the
                              transposed post-output domain)
  WQ/WP [128, 8, 1024] fp8e4, WK [128, 8, 256] fp8e4, WV [128, 8, 256] bf16
Output OUT [8, 128, SLOC] bf16: OUT[ds, p, q] = out[b, q0+q, ds*128+p]
(the post projection emits P^T, so the host transposes back).

Kernel: softmax kept with t on PSUM partitions. QT = Wq^T X^T (pre-scaled).
KT is stored twice (group g in partitions 0:64 of slot 2g and partitions
64:128 of slot 2g+1); the two heads of a pair run as concurrent 64-row
tile_position (0,0)/(64,0) matmuls. Q/K projections, PV, and the post
projection run fp8 DoubleRow (2 k-chunks per matmul). V carries a ones
column so the PV matmul emits the softmax denominator as row 64. exp on
ScalarE from 2-bank PSUM blocks straight to fp8 (no max subtraction:
|scores| <= ~2.8 so exp <= ~16, well inside e4m3 range). Head pairs are
software-pipelined as in the baseline. The post projection computes
P^T[D-chunk, q] so the residual add streams from XTR and each output slot
DMAs out as it completes.
"""

import math

import numpy as np
import ml_dtypes

BF16 = ml_dtypes.bfloat16
FP8 = ml_dtypes.float8_e4m3  # IEEE e4m3 (max 240) == TRN float8e4

B, S, D = 2, 2048, 1024
HEADS, GROUPS, E = 16, 4, 64
HPG = HEADS // GROUPS
NCORES = 8
CORES_PER_BATCH = NCORES // B
SLOC = B * S // NCORES
SCALE = 1.0 / math.sqrt(E)

_prog_cache = {}


def _build_program():
    from contextlib import ExitStack

    import concourse.bacc as bacc
    import concourse.tile as tile
    from concourse import mybir

    f32 = mybir.dt.float32
    b16 = mybir.dt.bfloat16
    i32 = mybir.dt.int32
    f8 = mybir.dt.float8e4
    f8e5 = mybir.dt.float8e5
    DR = mybir.MatmulPerfMode.DoubleRow
    Exp = mybir.ActivationFunctionType.Exp
    # Schraudolph fast-exp constants: exp(x) ~= bitcast_f32(int32(x*A + B))
    EXP_A = (1 << 23) / math.log(2.0)
    EXP_B = 127.0 * (1 << 23) - 366393.0

    nc = bacc.Bacc("TRN2", target_bir_lowering=False)

    xt_d = nc.dram_tensor("XT", [128, 8, S], f8, kind="ExternalInput")
    xtq_d = nc.dram_tensor("XTQ", [128, 8, SLOC], f8, kind="ExternalInput")
    xtr_d = nc.dram_tensor("XTR", [128, 8, SLOC], b16, kind="ExternalInput")
    wq_d = nc.dram_tensor("WQ", [128, 8, 1024], f8, kind="ExternalInput")
    wk_d = nc.dram_tensor("WK", [128, 8, 256], f8, kind="ExternalInput")
    wv_d = nc.dram_tensor("WV", [128, 8, 256], b16, kind="ExternalInput")
    wp_d = nc.dram_tensor("WP", [128, 8, 1024], f8, kind="ExternalInput")
    out_d = nc.dram_tensor("OUT", [8, 128, SLOC], b16, kind="ExternalOutput")

    with tile.TileContext(nc) as tc, ExitStack() as ctx:
        consts = ctx.enter_context(tc.tile_pool(name="consts", bufs=1))
        work = ctx.enter_context(tc.tile_pool(name="work", bufs=2))
        # PSUM (8 banks): scores 2x2 + pv 2x1 + pp(proj/post/bcast) 2x1
        psA = ctx.enter_context(tc.tile_pool(name="psA", bufs=2, space="PSUM"))
        psS = ctx.enter_context(tc.tile_pool(name="psS", bufs=2, space="PSUM"))
        psV = ctx.enter_context(tc.tile_pool(name="psV", bufs=1, space="PSUM"))

        xt = consts.tile([128, 8, S], f8)
        xtq = consts.tile([128, 8, SLOC], f8)
        xtr = consts.tile([128, 8, SLOC], b16)
        wq = consts.tile([128, 8, 1024], f8)
        wk = consts.tile([128, 8, 256], f8)
        wv = consts.tile([128, 8, 256], b16)
        wp = consts.tile([128, 8, 1024], f8)
        ktz = consts.tile([128, 8, S], b16)  # slot 2g: K^T[g] parts 0:64; slot 2g+1: parts 64:128
        vpr = consts.tile([128, 8, 2, 4, 68], f8)  # [t, tp, u, g, e+ones(64)]; 68 pads Ko step to 272B
        qt = consts.tile([128, 8, SLOC], b16)
        atn = consts.tile([128, 8, SLOC], f8e5)  # attn weights ~5e-4: e5m2 range
        pacc = consts.tile([128, 8, SLOC], f32)  # post round-1 partial (incl residual)
        e64 = consts.tile([128, 128], b16)  # row 64 = 1, else 0 (K=128 bcast)
        rbe = consts.tile([128, 512], b16)  # recip staging, rows != 64 stay 0
        rbo = consts.tile([128, 512], b16)
        warm = consts.tile([128, 8], f32)
        dumb = consts.tile([128, 512], b16)

        # HAM warmup first: keep the PE busy while input DMAs stream so
        # k_proj runs at 2.4 GHz instead of the cold 1.2 GHz default.
        nc.vector.memset(dumb[:], 0.0)
        for _ in range(10):
            wps = psA.tile([128, 512], f32, tag="pp")
            nc.tensor.matmul(wps[:], lhsT=dumb[:, 0:128], rhs=dumb[:], start=True, stop=True)

        nc.vector.memset(e64[:], 0.0)
        nc.vector.memset(e64[64:65, :], 1.0)
        nc.vector.memset(rbe[:], 0.0)
        nc.vector.memset(rbo[:], 0.0)
        with nc.allow_low_precision(reason="fp8 ones column"):
            nc.vector.memset(vpr[:, :, :, :, E:E + 1], 1.0)
        nc.vector.memset(warm[:], 0.0)
        nc.scalar.activation(warm[:], warm[:], Exp)  # exp table preload

        nc.sync.dma_start(out=wk[:], in_=wk_d[:])
        for t4 in range(4):
            sl = slice(t4 * 512, (t4 + 1) * 512)
            nc.sync.dma_start(out=xt[:, :, sl], in_=xt_d[:, :, sl])
        nc.sync.dma_start(out=xtq[:], in_=xtq_d[:])
        nc.sync.dma_start(out=wq[:], in_=wq_d[:])
        nc.sync.dma_start(out=wv[:], in_=wv_d[:])
        nc.sync.dma_start(out=xtr[:], in_=xtr_d[:])
        nc.sync.dma_start(out=wp[:], in_=wp_d[:])

        def k_proj(ec):
            for tb in range(4):
                ps = psA.tile([128, 512], f32, tag="pp")
                for kc in range(0, 8, 2):
                    nc.tensor.matmul(
                        ps[:],
                        lhsT=wk[:, kc:kc + 2, ec * 128:(ec + 1) * 128],
                        rhs=xt[:, kc:kc + 2, tb * 512:(tb + 1) * 512],
                        start=(kc == 0),
                        stop=(kc == 6),
                        perf_mode=DR,
                    )
                sl = slice(tb * 512, (tb + 1) * 512)
                ga, gb = 2 * ec, 2 * ec + 1
                nc.vector.tensor_copy(ktz[0:64, 2 * ga, sl], ps[0:64, :])
                nc.vector.tensor_copy(ktz[64:128, 2 * gb + 1, sl], ps[64:128, :])
                nc.gpsimd.dma_start(out=ktz[64:128, 2 * ga + 1, sl], in_=ktz[0:64, 2 * ga, sl])
                nc.gpsimd.dma_start(out=ktz[0:64, 2 * gb, sl], in_=ktz[64:128, 2 * gb + 1, sl])

        def q_proj(hc):
            ps = psA.tile([128, 512], f32, tag="pp")
            for kc in range(0, 8, 2):
                nc.tensor.matmul(
                    ps[:],
                    lhsT=wq[:, kc:kc + 2, hc * 128:(hc + 1) * 128],
                    rhs=xtq[:, kc:kc + 2, :],
                    start=(kc == 0),
                    stop=(kc == 6),
                    perf_mode=DR,
                )
            nc.vector.tensor_scalar_mul(qt[:, hc, :], ps, SCALE)

        def v_proj_tcc(tcc):
            ps = psA.tile([128, 256], f32, tag="pp")
            for kc in range(8):
                nc.tensor.matmul(
                    ps[:],
                    lhsT=xt[:, kc, tcc * 128:(tcc + 1) * 128],
                    rhs=wv[:, kc, :],
                    start=(kc == 0),
                    stop=(kc == 7),
                )
            tp, u = tcc // 2, tcc % 2
            with nc.allow_low_precision(reason="fp8 V for DoubleRow PV"):
                nc.vector.tensor_copy(
                    vpr[:, tp, u, :, 0:E], ps.rearrange("p (g e) -> p g e", g=4)
                )

        state = {}  # live psV tiles per pair: c -> (pve, pvo)

        def pv_mm(c, tp):
            g = c // 2
            if tp == 0:
                state[c] = (
                    psV.tile([E + 1, 512], f32, tag="pve", name="pve"),
                    psV.tile([E + 1, 512], f32, tag="pvo", name="pvo"),
                )
            pve, pvo = state[c]
            exd = exds.pop((c, tp))
            for j, pv in ((0, pve), (1, pvo)):
                nc.tensor.matmul(
                    pv[:],
                    lhsT=vpr[:, tp, :, g, 0:E + 1],
                    rhs=exd[:, :, j, :],
                    start=(tp == 0),
                    stop=(tp == 7),
                    perf_mode=DR,
                )

        aun = {}

        def pv_evict(c):
            """Copy A'+den to SBUF right after PV stop so the PSUM slots free
            early; the whole normalize chain then runs from SBUF."""
            pve, pvo = state.pop(c)
            te = work.tile([65, 512], b16, tag="aune", name="aune")
            to = work.tile([65, 512], b16, tag="auno", name="auno")
            nc.vector.tensor_copy(te[:], pve[:])
            nc.vector.tensor_copy(to[:], pvo[:])
            aun[c] = (te, to)

        def recips(c):
            te, to = aun[c]
            for t, rb in ((te, rbe), (to, rbo)):
                # spread the 512 denominators over 64 partitions so the DVE
                # reciprocal runs at 8 elements/lane instead of 512
                dsp = work.tile([64, 8], b16, tag="dsp")
                nc.gpsimd.dma_start(
                    out=dsp[:, None, :],
                    in_=t[64:65, :].rearrange("p (a b) -> p a b", a=64),
                )
                rsp = work.tile([64, 8], b16, tag="rsp")
                with nc.allow_low_precision(reason="bf16 softmax recip"):
                    nc.vector.reciprocal(rsp[:], dsp[:])
                nc.gpsimd.dma_start(
                    out=rb[64:65, :].rearrange("p (a b) -> p a b", a=64),
                    in_=rsp[:, None, :],
                )

        def norm_head(c, j):
            te, to = aun[c]
            t, rb = (te, rbe) if j == 0 else (to, rbo)
            bc = psA.tile([128, 512], f32, tag="pp")
            nc.tensor.matmul(bc[:], lhsT=e64[:], rhs=rb[:], start=True, stop=True)
            with nc.allow_low_precision(reason="fp8e5 attn weights"):
                if j == 0:
                    nc.vector.tensor_mul(atn[0:64, c, :], t[0:64, :], bc[0:64, :])
                else:
                    so = work.tile([64, 512], f8e5, tag="so")
                    nc.vector.tensor_mul(so[:], t[0:64, :], bc[0:64, :])
                    nc.gpsimd.dma_start(out=atn[64:128, c, :], in_=so[:])
                    aun.pop(c)

        def post_slot(rnd, ds):
            """P^T output slot ds (D columns ds*128..+128). Round 0 does
            he-chunks 0-3 (2 DR matmuls) + residual; rounds 1 and 2 do one
            he-chunk-pair each (4,5 / 6,7), accumulating into pacc; round 2
            also emits the output slot."""
            pp = psA.tile([128, 512], f32, tag="pp")
            ks = (0, 2) if rnd == 0 else (2 + 2 * rnd,)
            for k in ks:
                nc.tensor.matmul(
                    pp[:],
                    lhsT=wp[:, k:k + 2, ds * 128:(ds + 1) * 128],
                    rhs=atn[:, k:k + 2, :],
                    start=(k == ks[0]),
                    stop=(k == ks[-1]),
                    perf_mode=DR,
                )
            if rnd == 0:
                nc.vector.tensor_add(pacc[:, ds, :], pp[:], xtr[:, ds, :])
            elif rnd == 1:
                nc.vector.tensor_add(pacc[:, ds, :], pp[:], pacc[:, ds, :])
            else:
                ores = work.tile([128, SLOC], b16, tag="or")
                nc.vector.tensor_add(ores[:], pp[:], pacc[:, ds, :])
                nc.sync.dma_start(out=out_d[ds], in_=ores[:])

        exds = {}

        def pair_blocks(c):
            """One pipeline step: scores/exp for pair c (2-t-chunk groups so
            the concurrent 64-row tile pairs batch together), trailing PV for
            c, and the tail (PV-finish, normalize, post) of pair c-1."""
            g = c // 2
            for tcb in range(8):
                if c < 8:
                    if c == 0:
                        v_proj_tcc(2 * tcb)
                        v_proj_tcc(2 * tcb + 1)
                    tiles = []
                    for u in range(2):
                        tcc = 2 * tcb + u
                        p0 = psS.tile([128, 512], f32, tag="sc", name="ps0", bufs=4)
                        p1 = psS.tile([128, 512], f32, tag="sc", name="ps1", bufs=4)
                        tiles.append((tcc, p0, p1))
                    # all four 64-row tile-position matmuls adjacent:
                    # (0,0)/(64,0) pairs run concurrently on the PE
                    for tcc, p0, p1 in tiles:
                        csl = slice(tcc * 128, (tcc + 1) * 128)
                        nc.tensor.matmul(
                            p0[:],
                            lhsT=ktz[0:64, 2 * g, csl],
                            rhs=qt[0:64, c, :],
                            start=True,
                            stop=True,
                        )
                        nc.tensor.matmul(
                            p1[:],
                            lhsT=ktz[64:128, 2 * g + 1, csl],
                            rhs=qt[64:128, c, :],
                            start=True,
                            stop=True,
                        )
                    for tcc, p0, p1 in tiles:
                        tp, u = tcc // 2, tcc % 2
                        if u == 0:
                            exd = work.tile([128, 2, 2, 512], f8, tag="exd", bufs=4)
                            exds[(c, tp)] = exd
                        exd = exds[(c, tp)]
                        with nc.allow_low_precision(reason="fp8 softmax weights"):
                            nc.scalar.activation(exd[:, u, 0], p0[:], Exp)
                            if u == 1:
                                # hybrid exp: offload this half-chunk to
                                # DVE (Schraudolph fma->int32) + GpSimd
                                # (bitcast f32 -> fp8) to unload ScalarE
                                it = work.tile([128, 512], i32, tag="it32", bufs=2)
                                nc.vector.tensor_scalar(
                                    it[:], p1[:], EXP_A, EXP_B,
                                    op0=mybir.AluOpType.mult,
                                    op1=mybir.AluOpType.add,
                                )
                                nc.gpsimd.tensor_copy(exd[:, u, 1], it[:].bitcast(f32))
                            else:
                                nc.scalar.activation(exd[:, u, 1], p1[:], Exp)
                # tail of previous pair in fixed group slots
                if c > 0:
                    b = c - 1
                    if tcb == 0:
                        pv_mm(b, 6)
                    elif tcb == 1:
                        pv_mm(b, 7)
                        pv_evict(b)
                    elif tcb == 2:
                        recips(b)
                    elif tcb == 3:
                        norm_head(b, 0)
                    elif tcb == 4:
                        norm_head(b, 1)
                # post round 0 (he-chunks 0..3): pairs 0-3 all normalized
                # after pair 3's tail, i.e. from block c=4 tcb>=5
                if c == 4 and tcb >= 5:
                    post_slot(0, 2 * (tcb - 5))
                    post_slot(0, 2 * (tcb - 5) + 1)
                elif c == 5 and tcb == 0:
                    post_slot(0, 6)
                    post_slot(0, 7)
                # post round 1 (he-chunks 4,5): pairs 4,5 done after block 5
                if c == 6 and tcb >= 5:
                    post_slot(1, 2 * (tcb - 5))
                    post_slot(1, 2 * (tcb - 5) + 1)
                elif c == 7 and tcb == 0:
                    post_slot(1, 6)
                    post_slot(1, 7)
                # post round 2 (he-chunks 6,7): after pair 7's tail in drain
                if c == 8 and tcb >= 5:
                    for i in range((tcb - 5) * 3, min((tcb - 4) * 3, 8)):
                        post_slot(2, i)
                if c < 6 and tcb == 5:
                    q_proj(c + 2)
                # trailing PV for this pair (2 chunk-pairs behind)
                if c < 8 and tcb >= 2:
                    pv_mm(c, tcb - 2)

        k_proj(0)
        q_proj(0)
        pair_blocks(0)
        q_proj(1)
        pair_blocks(1)
        k_proj(1)
        for c in range(2, 8):
            pair_blocks(c)
        pair_blocks(8)  # drain: tail of pair 7, post round 2, output


    nc.compile()
    return nc


def get_program():
    if "nc" not in _prog_cache:
        _prog_cache["nc"] = _build_program()
    return _prog_cache["nc"]


def _chunk128(a):
    n = a.shape[1]
    return np.ascontiguousarray(a.reshape(8, 128, n).transpose(1, 0, 2))


def make_in_maps(X, Wq, Wk, Wv, Wpost):
    X = np.asarray(X, dtype=np.float32)
    wq_p = _chunk128(np.asarray(Wq, dtype=np.float32)).astype(FP8)
    wk_p = _chunk128(np.asarray(Wk, dtype=np.float32)).astype(FP8)
    wv_p = _chunk128(np.asarray(Wv, dtype=np.float32)).astype(BF16)
    wp_p = _chunk128(np.asarray(Wpost, dtype=np.float32)).astype(FP8)

    xt_b = []
    for b in range(B):
        xt_b.append(_chunk128(np.ascontiguousarray(X[b].T)))

    in_maps = []
    for core in range(NCORES):
        b = core // CORES_PER_BATCH
        q0 = (core % CORES_PER_BATCH) * SLOC
        xt = xt_b[b]
        xq = np.ascontiguousarray(xt[:, :, q0:q0 + SLOC])
        in_maps.append(
            {
                "XT": xt.astype(FP8),
                "XTQ": xq.astype(FP8),
                "XTR": xq.astype(BF16),
                "WQ": wq_p,
                "WK": wk_p,
                "WV": wv_p,
                "WP": wp_p,
            }
        )
    return in_maps


def sim_out(arr):
    """Core-0 output slice [SLOC, D] from the raw OUT tensor (for test.py sim)."""
    return np.asarray(arr).astype(np.float32).reshape(D, SLOC).T


def assemble_output(results):
    out = np.empty((B, S, D), dtype=np.float32)
    for core, r in enumerate(results):
        b = core // CORES_PER_BATCH
        q0 = (core % CORES_PER_BATCH) * SLOC
        # OUT [8, 128, SLOC] is P^T + X^T: D = ds*128 + p
        out[b, q0:q0 + SLOC] = (
            np.asarray(r["OUT"]).astype(np.float32).reshape(D, SLOC).T
        )
    return out


def kernel(X, Wq, Wk, Wv, Wpost, _trace=False):
    from concourse.bass_utils import run_bass_kernel_spmd

    nc = get_program()
    in_maps = make_in_maps(X, Wq, Wk, Wv, Wpost)
    res = run_bass_kernel_spmd(nc, in_maps, core_ids=list(range(NCORES)), trace=_trace)
    out = assemble_output(res.results)
    if _trace:
        return out, res
    return out


# revision 14
# speedup vs baseline: 1.3780x; 1.3780x over previous
"""GroupedQueryAttentionLayer on 8 trn2 NeuronCores (Bass/Tile, SPMD).

Sharding: data-parallel over query rows; no collectives. Core i handles
batch b = i//4, query rows q0 = (i%4)*512 .. +512. Each core recomputes
its batch's K/V projection (cheap vs. attention); outputs are disjoint
row-slices of the final [2, 2048, 1024].

Host-prepared per-core layouts (transposes/casts are part of sharding):
  XT   [128, 8, 2048] fp8e4 : X[b].T k-chunked (XT[p,c,t] = X[b,t,c*128+p])
  XTQ  [128, 8,  512] fp8e4 : XT columns q0..q0+512 (this core's queries)
  XTR  [128, 8,  512] bf16  : same as XTQ in bf16 (residual, added in the
                              transposed post-output domain)
  WQ/WP [128, 8, 1024] fp8e4, WK [128, 8, 256] fp8e4, WV [128, 8, 256] bf16
Output OUT [8, 128, SLOC] bf16: OUT[ds, p, q] = out[b, q0+q, ds*128+p]
(the post projection emits P^T, so the host transposes back).

Kernel: softmax kept with t on PSUM partitions. QT = Wq^T X^T (pre-scaled).
KT is stored twice (group g in partitions 0:64 of slot 2g and partitions
64:128 of slot 2g+1); the two heads of a pair run as concurrent 64-row
tile_position (0,0)/(64,0) matmuls. Q/K projections, PV, and the post
projection run fp8 DoubleRow (2 k-chunks per matmul). V carries a ones
column so the PV matmul emits the softmax denominator as row 64. exp on
ScalarE from 2-bank PSUM blocks straight to fp8 (no max subtraction:
|scores| <= ~2.8 so exp <= ~16, well inside e4m3 range). Head pairs are
software-pipelined as in the baseline. The post projection computes
P^T[D-chunk, q] so the residual add streams from XTR and each output slot
DMAs out as it completes.
"""

import math

import numpy as np
import ml_dtypes

BF16 = ml_dtypes.bfloat16
FP8 = ml_dtypes.float8_e4m3  # IEEE e4m3 (max 240) == TRN float8e4

B, S, D = 2, 2048, 1024
HEADS, GROUPS, E = 16, 4, 64
HPG = HEADS // GROUPS
NCORES = 8
CORES_PER_BATCH = NCORES // B
SLOC = B * S // NCORES
SCALE = 1.0 / math.sqrt(E)

_prog_cache = {}


def _build_program():
    from contextlib import ExitStack

    import concourse.bacc as bacc
    import concourse.tile as tile
    from concourse import mybir

    f32 = mybir.dt.float32
    b16 = mybir.dt.bfloat16
    i32 = mybir.dt.int32
    f8 = mybir.dt.float8e4
    f8e5 = mybir.dt.float8e5
    DR = mybir.MatmulPerfMode.DoubleRow
    Exp = mybir.ActivationFunctionType.Exp
    # Schraudolph fast-exp constants: exp(x) ~= bitcast_f32(int32(x*A + B))
    EXP_A = (1 << 23) / math.log(2.0)
    EXP_B = 127.0 * (1 << 23) - 366393.0

    nc = bacc.Bacc("TRN2", target_bir_lowering=False)

    xt_d = nc.dram_tensor("XT", [128, 8, S], f8, kind="ExternalInput")
    xtq_d = nc.dram_tensor("XTQ", [128, 8, SLOC], f8, kind="ExternalInput")
    xtr_d = nc.dram_tensor("XTR", [128, 8, SLOC], b16, kind="ExternalInput")
    wq_d = nc.dram_tensor("WQ", [128, 8, 1024], f8, kind="ExternalInput")
    wk_d = nc.dram_tensor("WK", [128, 8, 256], f8, kind="ExternalInput")
    wv_d = nc.dram_tensor("WV", [128, 8, 256], b16, kind="ExternalInput")
    wp_d = nc.dram_tensor("WP", [128, 8, 1024], f8, kind="ExternalInput")
    out_d = nc.dram_tensor("OUT", [8, 128, SLOC], b16, kind="ExternalOutput")

    with tile.TileContext(nc) as tc, ExitStack() as ctx:
        consts = ctx.enter_context(tc.tile_pool(name="consts", bufs=1))
        work = ctx.enter_context(tc.tile_pool(name="work", bufs=2))
        # PSUM (8 banks): scores 2x2 + pv 2x1 + pp(proj/post/bcast) 2x1
        psA = ctx.enter_context(tc.tile_pool(name="psA", bufs=2, space="PSUM"))
        psS = ctx.enter_context(tc.tile_pool(name="psS", bufs=2, space="PSUM"))
        psV = ctx.enter_context(tc.tile_pool(name="psV", bufs=1, space="PSUM"))

        xt = consts.tile([128, 8, S], f8)
        xtq = consts.tile([128, 8, SLOC], f8)
        xtr = consts.tile([128, 8, SLOC], b16)
        wq = consts.tile([128, 8, 1024], f8)
        wk = consts.tile([128, 8, 256], f8)
        wv = consts.tile([128, 8, 256], b16)
        wp = consts.tile([128, 8, 1024], f8)
        ktz = consts.tile([128, 8, S], b16)  # slot 2g: K^T[g] parts 0:64; slot 2g+1: parts 64:128
        vpr = consts.tile([128, 8, 2, 4, 68], f8)  # [t, tp, u, g, e+ones(64)]; 68 pads Ko step to 272B
        qt = consts.tile([128, 8, SLOC], b16)
        atn = consts.tile([128, 8, SLOC], f8e5)  # attn weights ~5e-4: e5m2 range
        pacc = consts.tile([128, 8, SLOC], f32)  # post round-1 partial (incl residual)
        e64 = consts.tile([128, 128], b16)  # row 64 = 1, else 0 (K=128 bcast)
        rbe = consts.tile([128, 512], b16)  # recip staging, rows != 64 stay 0
        rbo = consts.tile([128, 512], b16)
        warm = consts.tile([128, 8], f32)
        dumb = consts.tile([128, 512], b16)

        # HAM warmup first: keep the PE busy while input DMAs stream so
        # k_proj runs at 2.4 GHz instead of the cold 1.2 GHz default.
        nc.vector.memset(dumb[:], 0.0)
        for _ in range(10):
            wps = psA.tile([128, 512], f32, tag="pp")
            nc.tensor.matmul(wps[:], lhsT=dumb[:, 0:128], rhs=dumb[:], start=True, stop=True)

        nc.vector.memset(e64[:], 0.0)
        nc.vector.memset(e64[64:65, :], 1.0)
        nc.vector.memset(rbe[:], 0.0)
        nc.vector.memset(rbo[:], 0.0)
        with nc.allow_low_precision(reason="fp8 ones column"):
            nc.vector.memset(vpr[:, :, :, :, E:E + 1], 1.0)
        nc.vector.memset(warm[:], 0.0)
        nc.scalar.activation(warm[:], warm[:], Exp)  # exp table preload

        nc.sync.dma_start(out=wk[:], in_=wk_d[:])
        for t4 in range(4):
            sl = slice(t4 * 512, (t4 + 1) * 512)
            nc.sync.dma_start(out=xt[:, :, sl], in_=xt_d[:, :, sl])
        nc.sync.dma_start(out=xtq[:], in_=xtq_d[:])
        nc.sync.dma_start(out=wq[:], in_=wq_d[:])
        nc.sync.dma_start(out=wv[:], in_=wv_d[:])
        nc.sync.dma_start(out=xtr[:], in_=xtr_d[:])
        nc.sync.dma_start(out=wp[:], in_=wp_d[:])

        def k_proj(ec):
            for tb in range(4):
                ps = psA.tile([128, 512], f32, tag="pp")
                for kc in range(0, 8, 2):
                    nc.tensor.matmul(
                        ps[:],
                        lhsT=wk[:, kc:kc + 2, ec * 128:(ec + 1) * 128],
                        rhs=xt[:, kc:kc + 2, tb * 512:(tb + 1) * 512],
                        start=(kc == 0),
                        stop=(kc == 6),
                        perf_mode=DR,
                    )
                sl = slice(tb * 512, (tb + 1) * 512)
                ga, gb = 2 * ec, 2 * ec + 1
                nc.vector.tensor_copy(ktz[0:64, 2 * ga, sl], ps[0:64, :])
                nc.vector.tensor_copy(ktz[64:128, 2 * gb + 1, sl], ps[64:128, :])
                nc.gpsimd.dma_start(out=ktz[64:128, 2 * ga + 1, sl], in_=ktz[0:64, 2 * ga, sl])
                nc.gpsimd.dma_start(out=ktz[0:64, 2 * gb, sl], in_=ktz[64:128, 2 * gb + 1, sl])

        def q_proj(hc):
            ps = psA.tile([128, 512], f32, tag="pp")
            for kc in range(0, 8, 2):
                nc.tensor.matmul(
                    ps[:],
                    lhsT=wq[:, kc:kc + 2, hc * 128:(hc + 1) * 128],
                    rhs=xtq[:, kc:kc + 2, :],
                    start=(kc == 0),
                    stop=(kc == 6),
                    perf_mode=DR,
                )
            nc.vector.tensor_scalar_mul(qt[:, hc, :], ps, SCALE)

        def v_proj_tcc(tcc):
            ps = psA.tile([128, 256], f32, tag="pp")
            for kc in range(8):
                nc.tensor.matmul(
                    ps[:],
                    lhsT=xt[:, kc, tcc * 128:(tcc + 1) * 128],
                    rhs=wv[:, kc, :],
                    start=(kc == 0),
                    stop=(kc == 7),
                )
            tp, u = tcc // 2, tcc % 2
            with nc.allow_low_precision(reason="fp8 V for DoubleRow PV"):
                nc.vector.tensor_copy(
                    vpr[:, tp, u, :, 0:E], ps.rearrange("p (g e) -> p g e", g=4)
                )

        state = {}  # live psV tiles per pair: c -> (pve, pvo)

        def pv_mm(c, tp):
            g = c // 2
            if tp == 0:
                state[c] = (
                    psV.tile([E + 1, 512], f32, tag="pve", name="pve"),
                    psV.tile([E + 1, 512], f32, tag="pvo", name="pvo"),
                )
            pve, pvo = state[c]
            exd = exds.pop((c, tp))
            for j, pv in ((0, pve), (1, pvo)):
                nc.tensor.matmul(
                    pv[:],
                    lhsT=vpr[:, tp, :, g, 0:E + 1],
                    rhs=exd[:, :, j, :],
                    start=(tp == 0),
                    stop=(tp == 7),
                    perf_mode=DR,
                )

        aun = {}

        def pv_evict(c):
            """Copy A'+den to SBUF right after PV stop so the PSUM slots free
            early; the whole normalize chain then runs from SBUF."""
            pve, pvo = state.pop(c)
            te = work.tile([65, 512], b16, tag="aune", name="aune")
            to = work.tile([65, 512], b16, tag="auno", name="auno")
            nc.vector.tensor_copy(te[:], pve[:])
            nc.vector.tensor_copy(to[:], pvo[:])
            aun[c] = (te, to)

        def recips(c):
            te, to = aun[c]
            for t, rb in ((te, rbe), (to, rbo)):
                # spread the 512 denominators over 64 partitions so the DVE
                # reciprocal runs at 8 elements/lane instead of 512
                dsp = work.tile([64, 8], b16, tag="dsp")
                nc.gpsimd.dma_start(
                    out=dsp[:, None, :],
                    in_=t[64:65, :].rearrange("p (a b) -> p a b", a=64),
                )
                rsp = work.tile([64, 8], b16, tag="rsp")
                with nc.allow_low_precision(reason="bf16 softmax recip"):
                    nc.vector.reciprocal(rsp[:], dsp[:])
                nc.gpsimd.dma_start(
                    out=rb[64:65, :].rearrange("p (a b) -> p a b", a=64),
                    in_=rsp[:, None, :],
                )

        def norm_head(c, j):
            te, to = aun[c]
            t, rb = (te, rbe) if j == 0 else (to, rbo)
            bc = psA.tile([128, 512], f32, tag="pp")
            nc.tensor.matmul(bc[:], lhsT=e64[:], rhs=rb[:], start=True, stop=True)
            with nc.allow_low_precision(reason="fp8e5 attn weights"):
                if j == 0:
                    nc.vector.tensor_mul(atn[0:64, c, :], t[0:64, :], bc[0:64, :])
                else:
                    so = work.tile([64, 512], f8e5, tag="so")
                    nc.vector.tensor_mul(so[:], t[0:64, :], bc[0:64, :])
                    nc.gpsimd.dma_start(out=atn[64:128, c, :], in_=so[:])
                    aun.pop(c)

        def post_slot(rnd, ds):
            """P^T output slot ds (D columns ds*128..+128). Round 0 does
            he-chunks 0-3 (2 DR matmuls) + residual; rounds 1 and 2 do one
            he-chunk-pair each (4,5 / 6,7), accumulating into pacc; round 2
            also emits the output slot."""
            pp = psA.tile([128, 512], f32, tag="pp")
            ks = (0, 2) if rnd == 0 else (2 + 2 * rnd,)
            for k in ks:
                nc.tensor.matmul(
                    pp[:],
                    lhsT=wp[:, k:k + 2, ds * 128:(ds + 1) * 128],
                    rhs=atn[:, k:k + 2, :],
                    start=(k == ks[0]),
                    stop=(k == ks[-1]),
                    perf_mode=DR,
                )
            if rnd == 0:
                nc.vector.tensor_add(pacc[:, ds, :], pp[:], xtr[:, ds, :])
            elif rnd == 1:
                nc.vector.tensor_add(pacc[:, ds, :], pp[:], pacc[:, ds, :])
            else:
                ores = work.tile([128, SLOC], b16, tag="or")
                nc.vector.tensor_add(ores[:], pp[:], pacc[:, ds, :])
                nc.sync.dma_start(out=out_d[ds], in_=ores[:])

        exds = {}

        def pair_blocks(c):
            """One pipeline step: scores/exp for pair c (2-t-chunk groups so
            the concurrent 64-row tile pairs batch together), trailing PV for
            c, and the tail (PV-finish, normalize, post) of pair c-1."""
            g = c // 2
            for tcb in range(8):
                if c < 8:
                    if c == 0:
                        v_proj_tcc(2 * tcb)
                        v_proj_tcc(2 * tcb + 1)
                    tiles = []
                    for u in range(2):
                        tcc = 2 * tcb + u
                        p0 = psS.tile([128, 512], f32, tag="sc", name="ps0", bufs=4)
                        p1 = psS.tile([128, 512], f32, tag="sc", name="ps1", bufs=4)
                        tiles.append((tcc, p0, p1))
                    # all four 64-row tile-position matmuls adjacent:
                    # (0,0)/(64,0) pairs run concurrently on the PE
                    for tcc, p0, p1 in tiles:
                        csl = slice(tcc * 128, (tcc + 1) * 128)
                        nc.tensor.matmul(
                            p0[:],
                            lhsT=ktz[0:64, 2 * g, csl],
                            rhs=qt[0:64, c, :],
                            start=True,
                            stop=True,
                        )
                        nc.tensor.matmul(
                            p1[:],
                            lhsT=ktz[64:128, 2 * g + 1, csl],
                            rhs=qt[64:128, c, :],
                            start=True,
                            stop=True,
                        )
                    for tcc, p0, p1 in tiles:
                        tp, u = tcc // 2, tcc % 2
                        if u == 0:
                            exd = work.tile([128, 2, 2, 512], f8, tag="exd", bufs=4)
                            exds[(c, tp)] = exd
                        exd = exds[(c, tp)]
                        with nc.allow_low_precision(reason="fp8 softmax weights"):
                            nc.scalar.activation(exd[:, u, 0], p0[:], Exp)
                            if u == 1 and tcb % 4 != 3:
                                # hybrid exp: offload this half-chunk to the
                                # DVE (Schraudolph fma->int32, then bitcast
                                # f32 -> fp8 copy) to unload ScalarE
                                it = work.tile([128, 512], i32, tag="it32", bufs=2)
                                nc.vector.tensor_scalar(
                                    it[:], p1[:], EXP_A, EXP_B,
                                    op0=mybir.AluOpType.mult,
                                    op1=mybir.AluOpType.add,
                                )
                                nc.vector.tensor_copy(exd[:, u, 1], it[:].bitcast(f32))
                            else:
                                nc.scalar.activation(exd[:, u, 1], p1[:], Exp)
                # tail of previous pair in fixed group slots
                if c > 0:
                    b = c - 1
                    if tcb == 0:
                        pv_mm(b, 6)
                    elif tcb == 1:
                        pv_mm(b, 7)
                        pv_evict(b)
                    elif tcb == 2:
                        recips(b)
                    elif tcb == 3:
                        norm_head(b, 0)
                    elif tcb == 4:
                        norm_head(b, 1)
                # post round 0 (he-chunks 0..3): pairs 0-3 all normalized
                # after pair 3's tail, i.e. from block c=4 tcb>=5
                if c == 4 and tcb >= 5:
                    post_slot(0, 2 * (tcb - 5))
                    post_slot(0, 2 * (tcb - 5) + 1)
                elif c == 5 and tcb == 0:
                    post_slot(0, 6)
                    post_slot(0, 7)
                # post round 1 (he-chunks 4,5): pairs 4,5 done after block 5
                if c == 6 and tcb >= 5:
                    post_slot(1, 2 * (tcb - 5))
                    post_slot(1, 2 * (tcb - 5) + 1)
                elif c == 7 and tcb == 0:
                    post_slot(1, 6)
                    post_slot(1, 7)
                # post round 2 (he-chunks 6,7): after pair 7's tail in drain
                if c == 8 and tcb >= 5:
                    for i in range((tcb - 5) * 3, min((tcb - 4) * 3, 8)):
                        post_slot(2, i)
                if c < 6 and tcb == 5:
                    q_proj(c + 2)
                # trailing PV for this pair (2 chunk-pairs behind)
                if c < 8 and tcb >= 2:
                    pv_mm(c, tcb - 2)

        k_proj(0)
        q_proj(0)
        pair_blocks(0)
        q_proj(1)
        pair_blocks(1)
        k_proj(1)
        for c in range(2, 8):
            pair_blocks(c)
        pair_blocks(8)  # drain: tail of pair 7, post round 2, output


    nc.compile()
    return nc


def get_program():
    if "nc" not in _prog_cache:
        _prog_cache["nc"] = _build_program()
    return _prog_cache["nc"]


def _chunk128(a):
    n = a.shape[1]
    return np.ascontiguousarray(a.reshape(8, 128, n).transpose(1, 0, 2))


def make_in_maps(X, Wq, Wk, Wv, Wpost):
    X = np.asarray(X, dtype=np.float32)
    wq_p = _chunk128(np.asarray(Wq, dtype=np.float32)).astype(FP8)
    wk_p = _chunk128(np.asarray(Wk, dtype=np.float32)).astype(FP8)
    wv_p = _chunk128(np.asarray(Wv, dtype=np.float32)).astype(BF16)
    wp_p = _chunk128(np.asarray(Wpost, dtype=np.float32)).astype(FP8)

    xt_b = []
    for b in range(B):
        xt_b.append(_chunk128(np.ascontiguousarray(X[b].T)))

    in_maps = []
    for core in range(NCORES):
        b = core // CORES_PER_BATCH
        q0 = (core % CORES_PER_BATCH) * SLOC
        xt = xt_b[b]
        xq = np.ascontiguousarray(xt[:, :, q0:q0 + SLOC])
        in_maps.append(
            {
                "XT": xt.astype(FP8),
                "XTQ": xq.astype(FP8),
                "XTR": xq.astype(BF16),
                "WQ": wq_p,
                "WK": wk_p,
                "WV": wv_p,
                "WP": wp_p,
            }
        )
    return in_maps


def sim_out(arr):
    """Core-0 output slice [SLOC, D] from the raw OUT tensor (for test.py sim)."""
    return np.asarray(arr).astype(np.float32).reshape(D, SLOC).T


def assemble_output(results):
    out = np.empty((B, S, D), dtype=np.float32)
    for core, r in enumerate(results):
        b = core // CORES_PER_BATCH
        q0 = (core % CORES_PER_BATCH) * SLOC
        # OUT [8, 128, SLOC] is P^T + X^T: D = ds*128 + p
        out[b, q0:q0 + SLOC] = (
            np.asarray(r["OUT"]).astype(np.float32).reshape(D, SLOC).T
        )
    return out


def kernel(X, Wq, Wk, Wv, Wpost, _trace=False):
    from concourse.bass_utils import run_bass_kernel_spmd

    nc = get_program()
    in_maps = make_in_maps(X, Wq, Wk, Wv, Wpost)
    res = run_bass_kernel_spmd(nc, in_maps, core_ids=list(range(NCORES)), trace=_trace)
    out = assemble_output(res.results)
    if _trace:
        return out, res
    return out
